# revision 1
# baseline (speedup 1.0000x reference)
"""GATConv x2 + MLP head GNN, distributed over 8 Trainium2 NeuronCores.

Strategy (graph/data parallel per the sharding hint):
  - Partition nodes by destination across the 8 cores (12500 dst rows each);
    each core owns all edges whose destination lands in its range.
  - Host-side preprocessing is index manipulation + data movement only (no
    float arithmetic): per core, group incoming edges by destination, sort
    destinations by in-degree, build per-128-row-tile padded neighbor slot
    streams (per-tile max degree K_t, shared across cores so one SPMD
    program serves all 8), and "halo-exchange" = gather the per-slot source
    features from the full per-node arrays into a dense per-core stream.
  - Device (3 SPMD launches, all float math on device):
      L1: per-slot attention logits es1 = x_src.vsrc1 recomputed from the
          streamed x, fused leaky-relu/exp(+segment sum) and fused
          multiply+segment-reduce -> GAT1 output x2 rows, plus per-node
          es4/ed4 attention terms for layer 2.
      L2: streamed (x2_src | es4_src) slots -> 4-head GAT -> fc matmul +
          ReLU on TensorE -> BatchNorm partial stats.
      L3: BN finalize/normalize + 2 matmuls + sigmoid.
    Between launches the host only permutes/gathers device-computed arrays.
  - Padding slots carry a -1e30 bias so exp() -> 0; they contribute nothing.
"""

import os
import numpy as np

import concourse.bass as bass
import concourse.bacc as bacc
import concourse.tile as tile
from concourse import mybir
from concourse.bass_utils import run_bass_kernel_spmd
from concourse.masks import make_identity

FP = mybir.dt.float32
AF = mybir.ActivationFunctionType
OP = mybir.AluOpType

N_CORES = 8
SLOPE = 0.2
BN_EPS = 1e-5
NEG_BIG = -1.0e30

_PROG_CACHE = {}
LAST_RESULTS = []  # BassKernelResults of the most recent kernel() call


# --------------------------------------------------------------------------
# Host-side preprocessing (index manipulation only)
# --------------------------------------------------------------------------

def _preprocess(edge_index, n):
    src = np.asarray(edge_index[0], dtype=np.int64)
    dst = np.asarray(edge_index[1], dtype=np.int64)
    loops = np.arange(n, dtype=np.int64)
    src = np.concatenate([src, loops])
    dst = np.concatenate([dst, loops])

    assert n % N_CORES == 0
    R = n // N_CORES
    T = R // 128 + (1 if R % 128 else 0)
    R_pad = T * 128

    owner = dst // R
    per_core = []
    degs = []
    for c in range(N_CORES):
        m = owner == c
        s_c = src[m]
        d_loc = dst[m] - c * R
        deg = np.bincount(d_loc, minlength=R)
        row_of = np.argsort(-deg, kind="stable")
        per_core.append((s_c, d_loc, deg[row_of], row_of))
        degs.append(deg[row_of])

    tile_k = np.zeros(T, dtype=np.int64)
    for t in range(T):
        lo, hi = t * 128, min(t * 128 + 128, R)
        kmax = 1
        if hi > lo:
            for c in range(N_CORES):
                kmax = max(kmax, int(degs[c][lo:hi].max()))
        tile_k[t] = -(-max(kmax, 1) // 2) * 2
    tile_off = np.concatenate([[0], np.cumsum(tile_k * 128)])
    S = int(tile_off[-1])

    # per-core: slot -> source node (or -1 for pad), plus row -> node map
    slots_all = np.full((N_CORES, S), -1, dtype=np.int64)
    rows_node = np.empty((N_CORES, R), dtype=np.int64)
    for c in range(N_CORES):
        s_c, d_loc, deg_sorted, row_of = per_core[c]
        rank_of = np.empty(R, dtype=np.int64)
        rank_of[row_of] = np.arange(R)
        rows_node[c] = row_of + c * R

        erow = rank_of[d_loc]
        eorder = np.argsort(erow, kind="stable")
        erow_s = erow[eorder]
        esrc_s = s_c[eorder]
        row_start = np.concatenate([[0], np.cumsum(deg_sorted)])
        slot_no = np.arange(len(erow_s)) - row_start[erow_s]
        et = erow_s // 128
        ep = erow_s % 128
        pos = tile_off[et] + ep * tile_k[et] + slot_no
        slots_all[c][pos] = esrc_s
        # dummy rows (r >= R): slot 0 is a live zero-contribution slot
        for r in range(R, R_pad):
            t = r // 128
            p = r % 128
            slots_all[c][tile_off[t] + p * tile_k[t]] = -2
    return dict(
        R=R, R_pad=R_pad, T=T, S=S,
        tile_k=[int(k) for k in tile_k],
        tile_off=[int(o) for o in tile_off],
        slots=slots_all, rows_node=rows_node,
    )


def _make_stream(meta, pernode, es_pernode):
    """Build the per-core dense slot stream (plane-major within each tile).

    pernode: [n, F] float32 feature planes gathered by slot source.
    es_pernode: [n, H] attention-logit planes; pad slots get -1e30, dummy
    live slots (-2) get 0 features / 0 es.
    Layout per tile t, partition p: F+H planes of K_t contiguous floats.
    """
    F = pernode.shape[1]
    H = es_pernode.shape[1]
    n = pernode.shape[0]
    T = meta["T"]
    tile_k = meta["tile_k"]
    tile_off = meta["tile_off"]
    S = meta["S"]
    npl = F + H
    # cached permutation: stream position -> flat index into [S, npl]
    pkey = ("perm", npl)
    perm = meta.get(pkey)
    if perm is None:
        parts = []
        for t in range(T):
            K = tile_k[t]
            p_i = np.arange(128)[:, None, None]
            c_i = np.arange(npl)[None, :, None]
            k_i = np.arange(K)[None, None, :]
            parts.append(((tile_off[t] + p_i * K + k_i) * npl + c_i).reshape(-1))
        perm = np.concatenate(parts)
        meta[pkey] = perm
    out = np.empty((N_CORES, S * npl), dtype=np.float32)
    # feature table with two sentinel rows: n -> zeros/-1e30 (pad),
    # n+1 -> zeros/0 (dummy live slot)
    ptab = np.vstack([pernode, np.zeros((2, F), np.float32)])
    etab = np.vstack([es_pernode, np.full((1, H), NEG_BIG, np.float32),
                      np.zeros((1, H), np.float32)])
    for c in range(N_CORES):
        sl = meta["slots"][c].copy()
        sl[sl == -1] = n
        sl[sl == -2] = n + 1
        both = np.concatenate([ptab[sl], etab[sl]], axis=1)  # [S, npl]
        out[c] = both.reshape(-1)[perm]
    return out


# --------------------------------------------------------------------------
# Device program pieces
# --------------------------------------------------------------------------

def _edge_phase(nc, tc, pool, meta, g_dram, ed_rows, nch, nheads,
                es_from_x, vsrc=None):
    """GAT edge aggregation over the dense slot stream.

    g_dram: stream input [S*(nch+nheads... )] (planes per tile).
    ed_rows: SBUF [128, T, nheads] per-row dst logits.
    es_from_x: if True, stream planes are [x(nch) | maskbias(1)] and es is
      recomputed per slot from vsrc [128, nch]; else planes are
      [x(nch) | es(nheads)] with pad -1e30 baked in.
    Returns (M_all [128, T, nheads*nch], s_all [128, T*nheads]).
    """
    T = meta["T"]
    tile_k = meta["tile_k"]
    tile_off = meta["tile_off"]
    npl = nch + (1 if es_from_x else nheads)

    M_all = pool.tile([128, T, nheads * nch], FP)
    s_all = pool.tile([128, T * nheads], FP)

    with tc.tile_pool(name="edge", bufs=3) as ep, \
         tc.tile_pool(name="edge_t", bufs=3) as tp:
        for t in range(T):
            K = tile_k[t]
            G = ep.tile([128, npl, K], FP, tag="G")
            nc.sync.dma_start(out=G[:], in_=g_dram.ap()[
                tile_off[t] * npl:tile_off[t + 1] * npl].rearrange(
                "(p c k) -> p c k", p=128, c=npl))
            if es_from_x:
                es = tp.tile([128, 1, K], FP, tag="es")
                nc.vector.tensor_scalar(
                    out=es[:], in0=G[:, 0:1, :], scalar1=vsrc[:, 0:1],
                    scalar2=None, op0=OP.mult)
                for f in (1, 2):
                    nc.vector.scalar_tensor_tensor(
                        out=es[:], in0=G[:, f:f + 1, :],
                        scalar=vsrc[:, f:f + 1], in1=es[:],
                        op0=OP.mult, op1=OP.add)
            for h in range(nheads):
                tb = tp.tile([128, 1, K], FP, tag="tb")
                if es_from_x:
                    # tb = es + ed[row] + maskbias
                    nc.vector.scalar_tensor_tensor(
                        out=tb[:], in0=es[:], scalar=ed_rows[:, t, h:h + 1],
                        in1=G[:, nch:nch + 1, :], op0=OP.add, op1=OP.add)
                else:
                    nc.vector.tensor_scalar(
                        out=tb[:], in0=G[:, nch + h:nch + h + 1, :],
                        scalar1=ed_rows[:, t, h:h + 1], scalar2=None,
                        op0=OP.add)
                lr = tp.tile([128, 1, K], FP, tag="lr")
                nc.vector.scalar_tensor_tensor(
                    out=lr[:], in0=tb[:], scalar=SLOPE, in1=tb[:],
                    op0=OP.mult, op1=OP.max)
                e = tp.tile([128, 1, K], FP, tag="e")
                nc.scalar.activation(
                    out=e[:], in_=lr[:], func=AF.Exp,
                    accum_out=s_all[:, t * nheads + h:t * nheads + h + 1])
                for f in range(nch):
                    tr = tp.tile([128, 1, K], FP, tag="tr")
                    nc.vector.scalar_tensor_tensor(
                        out=tr[:], in0=e[:], scalar=1.0, in1=G[:, f:f + 1, :],
                        op0=OP.bypass, op1=OP.mult,
                        accum_out=M_all[:, t, h * nch + f:h * nch + f + 1])
    return M_all, s_all


def _gat_out(nc, tc, pool, meta, M_all, s_all, wb, bb, nch, nheads, out_sb,
             relu=True):
    """out_sb[:, :, hc] = relu((sum_f M[h,f] W[f,hc]) / s_h + b[hc])."""
    T = meta["T"]
    rn = pool.tile([128, T * nheads], FP)
    nc.vector.reciprocal(rn[:], s_all[:])
    rn3 = rn[:].rearrange("p (t h) -> p t h", h=nheads)
    wv = wb[:].rearrange("p (f hc) -> p f hc", f=nch)
    with tc.tile_pool(name="gatout", bufs=2) as gp:
        for h in range(nheads):
            for c in range(nch):
                hc = h * nch + c
                t1 = gp.tile([128, T], FP, tag="t1")
                for f in range(nch):
                    src = M_all[:, :, h * nch + f:h * nch + f + 1].rearrange(
                        "p t c -> p (t c)")
                    if f == 0:
                        nc.vector.tensor_scalar(
                            out=t1[:], in0=src, scalar1=wv[:, f, hc:hc + 1],
                            scalar2=None, op0=OP.mult)
                    else:
                        nc.vector.scalar_tensor_tensor(
                            out=t1[:], in0=src, scalar=wv[:, f, hc:hc + 1],
                            in1=t1[:], op0=OP.mult, op1=OP.add)
                t2 = gp.tile([128, T], FP, tag="t2")
                nc.vector.tensor_tensor(
                    out=t2[:], in0=t1[:],
                    in1=rn3[:, :, h:h + 1].rearrange("p t h -> p (t h)"),
                    op=OP.mult)
                nc.scalar.activation(
                    out=out_sb[:, :, hc:hc + 1].rearrange("p t c -> p (t c)"),
                    in_=t2[:], func=AF.Relu if relu else AF.Identity,
                    bias=bb[:, hc:hc + 1], scale=1.0)


def _vpair(nc, pool, wb, ab, nch, nheads, name):
    """v[f,h] = sum_c W[f, h*nch+c] * a[h*nch+c]  -> tile [128, nch, nheads]."""
    wv = wb[:].rearrange("p (f h c) -> p f h c", f=nch, h=nheads)
    vt = pool.tile([128, nch, nheads], FP, name=name)
    for h in range(nheads):
        for cc in range(nch):
            o = vt[:, :, h:h + 1].rearrange("p f h -> p (f h)")
            if cc == 0:
                nc.vector.tensor_scalar(
                    out=o, in0=wv[:, :, h, cc],
                    scalar1=ab[:, h * nch + cc:h * nch + cc + 1],
                    scalar2=None, op0=OP.mult)
            else:
                nc.vector.scalar_tensor_tensor(
                    out=o, in0=wv[:, :, h, cc],
                    scalar=ab[:, h * nch + cc:h * nch + cc + 1],
                    in1=o, op0=OP.mult, op1=OP.add)
    return vt


def _rows_affine(nc, tc, pool, meta, xr, vt, nch, nheads, name):
    """out[p, t, h] = sum_f xr[p, t, f] * vt[f, h] over row tiles."""
    T = meta["T"]
    out = pool.tile([128, T, nheads], FP, name=name)
    for h in range(nheads):
        o = out[:, :, h:h + 1].rearrange("p t h -> p (t h)")
        for f in range(nch):
            src = xr[:, :, f:f + 1].rearrange("p t f -> p (t f)")
            if f == 0:
                nc.vector.tensor_scalar(
                    out=o, in0=src, scalar1=vt[:, f, h:h + 1],
                    scalar2=None, op0=OP.mult)
            else:
                nc.vector.scalar_tensor_tensor(
                    out=o, in0=src, scalar=vt[:, f, h:h + 1],
                    in1=o, op0=OP.mult, op1=OP.add)
    return out


def _ld(nc, pool, dram, shape, name):
    t = pool.tile(shape, FP, name=name)
    nc.sync.dma_start(out=t[:], in_=dram.ap())
    return t


def _build_l1(meta):
    nc = bacc.Bacc("TRN2", target_bir_lowering=False, debug=False,
                   num_devices=N_CORES)
    T, S, R_pad = meta["T"], meta["S"], meta["R_pad"]
    g1 = nc.dram_tensor("g1", [S * 4], FP, kind="ExternalInput")
    xr_i = nc.dram_tensor("xr", [R_pad, 3], FP, kind="ExternalInput")
    w1b = nc.dram_tensor("w1b", [128, 9], FP, kind="ExternalInput")
    as1 = nc.dram_tensor("as1", [128, 3], FP, kind="ExternalInput")
    ad1 = nc.dram_tensor("ad1", [128, 3], FP, kind="ExternalInput")
    b1b = nc.dram_tensor("b1b", [128, 3], FP, kind="ExternalInput")
    w2b = nc.dram_tensor("w2b", [128, 36], FP, kind="ExternalInput")
    as2 = nc.dram_tensor("as2", [128, 12], FP, kind="ExternalInput")
    ad2 = nc.dram_tensor("ad2", [128, 12], FP, kind="ExternalInput")
    x2o = nc.dram_tensor("x2", [R_pad, 3], FP, kind="ExternalOutput")
    es4o = nc.dram_tensor("es4", [R_pad, 4], FP, kind="ExternalOutput")
    ed4o = nc.dram_tensor("ed4", [R_pad, 4], FP, kind="ExternalOutput")

    with tile.TileContext(nc) as tc:
        with tc.tile_pool(name="persist", bufs=1) as pool:
            wts = _ld(nc, pool, w1b, [128, 9], "wts")
            a_s = _ld(nc, pool, as1, [128, 3], "a_s")
            a_d = _ld(nc, pool, ad1, [128, 3], "a_d")
            b_b = _ld(nc, pool, b1b, [128, 3], "b_b")
            wts2 = _ld(nc, pool, w2b, [128, 36], "wts2")
            a_s2 = _ld(nc, pool, as2, [128, 12], "a_s2")
            a_d2 = _ld(nc, pool, ad2, [128, 12], "a_d2")
            xr = pool.tile([128, T, 3], FP)
            nc.sync.dma_start(out=xr[:], in_=xr_i.ap().rearrange(
                "(t p) c -> p t c", p=128))

            vs1 = _vpair(nc, pool, wts, a_s, 3, 1, "vs1")
            vd1 = _vpair(nc, pool, wts, a_d, 3, 1, "vd1")
            ed_rows = _rows_affine(nc, tc, pool, meta, xr, vd1, 3, 1, "edr")
            vsrc = vs1[:, :, 0]   # [128, 3]

            M_all, s_all = _edge_phase(nc, tc, pool, meta, g1, ed_rows,
                                       3, 1, True, vsrc=vsrc)
            x2t = pool.tile([128, T, 3], FP)
            _gat_out(nc, tc, pool, meta, M_all, s_all, wts, b_b, 3, 1, x2t)
            nc.sync.dma_start(
                out=x2o.ap().rearrange("(t p) c -> p t c", p=128), in_=x2t[:])

            # layer-2 per-node attention terms from x2 rows
            vs2 = _vpair(nc, pool, wts2, a_s2, 3, 4, "vs2")
            vd2 = _vpair(nc, pool, wts2, a_d2, 3, 4, "vd2")
            es4 = _rows_affine(nc, tc, pool, meta, x2t, vs2, 3, 4, "es4r")
            ed4 = _rows_affine(nc, tc, pool, meta, x2t, vd2, 3, 4, "ed4r")
            nc.sync.dma_start(
                out=es4o.ap().rearrange("(t p) c -> p t c", p=128), in_=es4[:])
            nc.sync.dma_start(
                out=ed4o.ap().rearrange("(t p) c -> p t c", p=128), in_=ed4[:])
    nc.compile()
    return nc


def _build_l2(meta, n):
    nc = bacc.Bacc("TRN2", target_bir_lowering=False, debug=False,
                   num_devices=N_CORES)
    T, S, R_pad, R = meta["T"], meta["S"], meta["R_pad"], meta["R"]
    g2 = nc.dram_tensor("g2", [S * 7], FP, kind="ExternalInput")
    ed4i = nc.dram_tensor("ed4", [R_pad, 4], FP, kind="ExternalInput")
    w2b = nc.dram_tensor("w2b", [128, 36], FP, kind="ExternalInput")
    b2b = nc.dram_tensor("b2b", [128, 12], FP, kind="ExternalInput")
    fcw = nc.dram_tensor("fcw", [12, 128], FP, kind="ExternalInput")
    fcb = nc.dram_tensor("fcb", [128, 1], FP, kind="ExternalInput")
    h3o = nc.dram_tensor("h3", [128, R_pad], FP, kind="ExternalOutput")
    sto = nc.dram_tensor("bnstat", [128, 2], FP, kind="ExternalOutput")

    with tile.TileContext(nc) as tc:
        with tc.tile_pool(name="persist", bufs=1) as pool:
            wts = _ld(nc, pool, w2b, [128, 36], "wts")
            b_b = _ld(nc, pool, b2b, [128, 12], "b_b")
            fcw_s = _ld(nc, pool, fcw, [12, 128], "fcw_s")
            fcb_s = _ld(nc, pool, fcb, [128, 1], "fcb_s")
            ed_rows = pool.tile([128, T, 4], FP)
            nc.sync.dma_start(out=ed_rows[:], in_=ed4i.ap().rearrange(
                "(t p) c -> p t c", p=128))

            M_all, s_all = _edge_phase(nc, tc, pool, meta, g2, ed_rows,
                                       3, 4, False)
            h2t = pool.tile([128, T, 12], FP)
            _gat_out(nc, tc, pool, meta, M_all, s_all, wts, b_b, 3, 4, h2t)

            ident = pool.tile([128, 128], FP)
            make_identity(nc, ident[:])
            h2T = pool.tile([12, R_pad], FP)
            with tc.tile_pool(name="tpsum", bufs=4, space="PSUM") as tps:
                for t in range(T):
                    ps = tps.tile([12, 128], FP, tag="ps")
                    nc.tensor.transpose(
                        out=ps[:], in_=h2t[:, t, :], identity=ident[:])
                    nc.scalar.copy(out=h2T[:, 128 * t:128 * (t + 1)],
                                   in_=ps[:])

            h3T = pool.tile([128, R_pad], FP)
            chunks = [(j, min(j + 512, R_pad)) for j in range(0, R_pad, 512)]
            with tc.tile_pool(name="fcpsum", bufs=4, space="PSUM") as fps:
                for (j0, j1) in chunks:
                    ps = fps.tile([128, j1 - j0], FP, tag="fc")
                    nc.tensor.matmul(ps[:], lhsT=fcw_s[:], rhs=h2T[:, j0:j1],
                                     start=True, stop=True)
                    nc.scalar.activation(out=h3T[:, j0:j1], in_=ps[:],
                                         func=AF.Relu, bias=fcb_s[:, 0:1],
                                         scale=1.0)
            nc.sync.dma_start(out=h3o.ap(), in_=h3T[:])

            st_chunks = [(j, min(j + 512, R)) for j in range(0, R, 512)]
            ns = len(st_chunks)
            sums = pool.tile([128, ns], FP)
            sqs = pool.tile([128, ns], FP)
            with tc.tile_pool(name="stat", bufs=2) as sp:
                for i, (j0, j1) in enumerate(st_chunks):
                    tr = sp.tile([128, 512], FP, tag="str")
                    nc.scalar.activation(out=tr[:, 0:j1 - j0],
                                         in_=h3T[:, j0:j1], func=AF.Copy,
                                         accum_out=sums[:, i:i + 1])
                    tr2 = sp.tile([128, 512], FP, tag="str2")
                    nc.vector.scalar_tensor_tensor(
                        out=tr2[:, 0:j1 - j0], in0=h3T[:, j0:j1], scalar=1.0,
                        in1=h3T[:, j0:j1], op0=OP.bypass, op1=OP.mult,
                        accum_out=sqs[:, i:i + 1])
            st2 = pool.tile([128, 2], FP)
            nc.vector.tensor_reduce(out=st2[:, 0:1], in_=sums[:],
                                    axis=mybir.AxisListType.X, op=OP.add)
            nc.vector.tensor_reduce(out=st2[:, 1:2], in_=sqs[:],
                                    axis=mybir.AxisListType.X, op=OP.add)
            nc.sync.dma_start(out=sto.ap(), in_=st2[:])
    nc.compile()
    return nc


def _build_l3(meta, n):
    nc = bacc.Bacc("TRN2", target_bir_lowering=False, debug=False,
                   num_devices=N_CORES)
    R_pad = meta["R_pad"]
    h3i = nc.dram_tensor("h3", [128, R_pad], FP, kind="ExternalInput")
    sti = nc.dram_tensor("bnstats", [128, 16], FP, kind="ExternalInput")
    bng = nc.dram_tensor("bng", [128, 1], FP, kind="ExternalInput")
    bnb = nc.dram_tensor("bnb", [128, 1], FP, kind="ExternalInput")
    l2w = nc.dram_tensor("l2w", [128, 64], FP, kind="ExternalInput")
    l2b = nc.dram_tensor("l2b", [64, 1], FP, kind="ExternalInput")
    ow = nc.dram_tensor("ow", [64, 6], FP, kind="ExternalInput")
    ob = nc.dram_tensor("ob", [6, 1], FP, kind="ExternalInput")
    out = nc.dram_tensor("out", [R_pad, 6], FP, kind="ExternalOutput")

    with tile.TileContext(nc) as tc:
        with tc.tile_pool(name="persist", bufs=1) as pool, \
             tc.tile_pool(name="psum", bufs=4, space="PSUM") as pp:
            h3s = pool.tile([128, R_pad], FP)
            q = R_pad // 4
            for j in range(0, R_pad, q):
                nc.sync.dma_start(out=h3s[:, j:j + q],
                                  in_=h3i.ap()[:, j:j + q])
            sts = _ld(nc, pool, sti, [128, 16], "sts")
            bng_s = _ld(nc, pool, bng, [128, 1], "bng_s")
            bnb_s = _ld(nc, pool, bnb, [128, 1], "bnb_s")
            l2w_s = _ld(nc, pool, l2w, [128, 64], "l2w_s")
            l2b_s = _ld(nc, pool, l2b, [64, 1], "l2b_s")
            ow_s = _ld(nc, pool, ow, [64, 6], "ow_s")
            ob_s = _ld(nc, pool, ob, [6, 1], "ob_s")

            red = pool.tile([128, 2], FP)
            nc.vector.tensor_reduce(
                out=red[:], in_=sts[:].rearrange("p (s c) -> p s c", s=2),
                axis=mybir.AxisListType.X, op=OP.add)
            mu = pool.tile([128, 1], FP)
            nc.vector.tensor_scalar(out=mu[:], in0=red[:, 0:1],
                                    scalar1=1.0 / n, scalar2=None, op0=OP.mult)
            m2 = pool.tile([128, 1], FP)
            nc.vector.tensor_scalar(out=m2[:], in0=red[:, 1:2],
                                    scalar1=1.0 / n, scalar2=None, op0=OP.mult)
            var = pool.tile([128, 1], FP)
            nc.vector.tensor_tensor(out=var[:], in0=mu[:], in1=mu[:],
                                    op=OP.mult)
            nc.vector.tensor_tensor(out=var[:], in0=m2[:], in1=var[:],
                                    op=OP.subtract)
            epsb = pool.tile([128, 1], FP)
            nc.vector.memset(epsb[:], BN_EPS)
            sd = pool.tile([128, 1], FP)
            nc.scalar.activation(out=sd[:], in_=var[:], func=AF.Sqrt,
                                 bias=epsb[:], scale=1.0)
            rsig = pool.tile([128, 1], FP)
            nc.vector.reciprocal(rsig[:], sd[:])
            scale = pool.tile([128, 1], FP)
            nc.vector.tensor_tensor(out=scale[:], in0=bng_s[:], in1=rsig[:],
                                    op=OP.mult)
            shift = pool.tile([128, 1], FP)
            nc.vector.tensor_tensor(out=shift[:], in0=mu[:], in1=scale[:],
                                    op=OP.mult)
            nc.vector.tensor_tensor(out=shift[:], in0=bnb_s[:], in1=shift[:],
                                    op=OP.subtract)
            # chunked BN-apply interleaved with both matmul stages so the
            # DVE normalize, PE matmuls and ACT bias/sigmoid pipeline overlap
            hbn = pool.tile([128, R_pad], FP)
            h4T = pool.tile([64, R_pad], FP)
            outT = pool.tile([6, R_pad], FP)
            chunks = [(j, min(j + 512, R_pad)) for j in range(0, R_pad, 512)]
            for (j0, j1) in chunks:
                nc.vector.tensor_scalar(out=hbn[:, j0:j1], in0=h3s[:, j0:j1],
                                        scalar1=scale[:], scalar2=shift[:],
                                        op0=OP.mult, op1=OP.add)
                ps = pp.tile([64, j1 - j0], FP, tag="l2")
                nc.tensor.matmul(ps[:], lhsT=l2w_s[:], rhs=hbn[:, j0:j1],
                                 start=True, stop=True)
                nc.scalar.activation(out=h4T[:, j0:j1], in_=ps[:],
                                     func=AF.Identity, bias=l2b_s[:, 0:1],
                                     scale=1.0)
                ps2 = pp.tile([6, j1 - j0], FP, tag="out")
                nc.tensor.matmul(ps2[:], lhsT=ow_s[:], rhs=h4T[:, j0:j1],
                                 start=True, stop=True)
                nc.scalar.activation(out=outT[:, j0:j1], in_=ps2[:],
                                     func=AF.Sigmoid, bias=ob_s[:, 0:1],
                                     scale=1.0)
            nc.sync.dma_start(out=out.ap().rearrange("r c -> c r"),
                              in_=outT[:])
    nc.compile()
    return nc


# --------------------------------------------------------------------------
# Orchestration
# --------------------------------------------------------------------------

def _bcast(a, cols):
    return np.ascontiguousarray(
        np.broadcast_to(np.asarray(a, np.float32).reshape(1, -1), (128, cols)))


LAUNCH_WALL = []


def _run(nc, in_maps, trace=False):
    import time as _t
    t0 = _t.perf_counter()
    res = run_bass_kernel_spmd(nc, in_maps, list(range(N_CORES)))
    LAUNCH_WALL.append(_t.perf_counter() - t0)
    LAST_RESULTS.append(res)
    return res.results


def _rows_to_pernode(meta, arrs):
    """arrs: list per core of [R_pad, F] row arrays -> [n, F] per-node."""
    R = meta["R"]
    F = arrs[0].shape[1]
    n = R * N_CORES
    out = np.empty((n, F), np.float32)
    for c in range(N_CORES):
        out[meta["rows_node"][c]] = arrs[c][:R]
    return out


def kernel(x, edge_index, W1, a_src1, a_dst1, b1, W2, a_src2, a_dst2, b2,
           fc_W, fc_b, bn_g, bn_b, l2_W, l2_b, out_W, out_b):
    global LAST_RESULTS
    LAST_RESULTS = []
    x = np.asarray(x, np.float32)
    n = x.shape[0]
    ekey = (n, np.asarray(edge_index).shape[1])
    meta = _PROG_CACHE.get(("meta", ekey))
    fp = np.asarray(edge_index)[:, :: max(1, ekey[1] // 64)]
    if meta is None or not np.array_equal(meta["_fp"], fp):
        meta = _preprocess(np.asarray(edge_index), n)
        meta["_fp"] = fp.copy()
        _PROG_CACHE.clear()
        _PROG_CACHE[("meta", ekey)] = meta

    R, R_pad = meta["R"], meta["R_pad"]
    if ("l1", ekey) not in _PROG_CACHE:
        _PROG_CACHE[("l1", ekey)] = _build_l1(meta)
        _PROG_CACHE[("l2", ekey)] = _build_l2(meta, n)
        _PROG_CACHE[("l3", ekey)] = _build_l3(meta, n)

    # ---- launch 1: stream = [x planes (3) | mask-bias plane (1)]
    # mask bias: 0 for live slots, -1e30 for padding (exp -> 0)
    g1 = _make_stream(meta, x, np.zeros((n, 1), np.float32))

    xr_all = []
    for c in range(N_CORES):
        xr = np.zeros((R_pad, 3), np.float32)
        xr[:R] = x[meta["rows_node"][c]]
        xr_all.append(xr)

    in_maps = []
    for c in range(N_CORES):
        in_maps.append(dict(
            g1=g1[c], xr=xr_all[c],
            w1b=_bcast(W1, 9), as1=_bcast(a_src1, 3), ad1=_bcast(a_dst1, 3),
            b1b=_bcast(b1, 3), w2b=_bcast(W2, 36), as2=_bcast(a_src2, 12),
            ad2=_bcast(a_dst2, 12)))
    r1 = _run(_PROG_CACHE[("l1", ekey)], in_maps)

    x2_pernode = _rows_to_pernode(meta, [r1[c]["x2"] for c in range(N_CORES)])
    es4_pernode = _rows_to_pernode(meta,
                                   [r1[c]["es4"] for c in range(N_CORES)])

    # ---- launch 2: stream = [x2 planes | es4 planes]
    g2 = _make_stream(meta, x2_pernode, es4_pernode)
    in_maps = []
    for c in range(N_CORES):
        in_maps.append(dict(
            g2=g2[c], ed4=np.ascontiguousarray(r1[c]["ed4"]),
            w2b=_bcast(W2, 36), b2b=_bcast(b2, 12),
            fcw=np.asarray(fc_W, np.float32),
            fcb=np.asarray(fc_b, np.float32).reshape(128, 1)))
    r2 = _run(_PROG_CACHE[("l2", ekey)], in_maps)

    stats = np.zeros((128, 16), np.float32)
    for c in range(N_CORES):
        stats[:, c] = r2[c]["bnstat"][:, 0]
        stats[:, 8 + c] = r2[c]["bnstat"][:, 1]

    # ---- launch 3
    in_maps = []
    for c in range(N_CORES):
        in_maps.append(dict(
            h3=r2[c]["h3"], bnstats=stats,
            bng=np.asarray(bn_g, np.float32).reshape(128, 1),
            bnb=np.asarray(bn_b, np.float32).reshape(128, 1),
            l2w=np.asarray(l2_W, np.float32),
            l2b=np.asarray(l2_b, np.float32).reshape(64, 1),
            ow=np.asarray(out_W, np.float32),
            ob=np.asarray(out_b, np.float32).reshape(6, 1)))
    r3 = _run(_PROG_CACHE[("l3", ekey)], in_maps)

    out = np.zeros((n, 6), np.float32)
    for c in range(N_CORES):
        out[meta["rows_node"][c]] = r3[c]["out"][:R]
    return out

